# revision 12
# baseline (speedup 1.0000x reference)
"""SSD DecodeLayer (box decode + per-anchor class max/argmax) on 8 trn2 cores.

Data-parallel over batch: each of the 8 cores processes 16 of the 128 batches.
Layout per core: anchors of one batch split as 8732 = 118 partitions x 74
anchors; tiles cover BPT=4 batches -> [118, 296 anchors, 25 ch] per tile.

Per-anchor math (ch = 4 loc + 21 classes):
  scores  = max_j cls_j                      (f32 reduce over class axis)
  eq_j    = cls_j >= scores                  (bf16 0/1)
  r       = max_j eq_j * (21-j)              (bf16; first-occurrence argmax)
  classes = 21 - r, valid = sign(21 - r), num = per-batch sum(valid)
  boxes   = clip(xy -+ bwh/2) with xy = loc01*wh + cxcy,
            bwh/2 = exp(loc23) * wh/2
"""

import numpy as np
import ml_dtypes

B, N, NCH = 128, 8732, 25
NCLS = 21          # 20 classes + background
NCORES = 8
BPC = B // NCORES  # batches per core
P, APB = 118, 74   # 118 * 74 == 8732
BPT = 4            # batches per tile
NTILES = BPC // BPT
AT = BPT * APB     # anchors per partition per tile
CP = NCLS + 1      # padded class width (even, for bf16 2x mode)

ALL_BLOCKS = ("scores", "argmax", "decode", "num")

_CACHE = {}


def build_program(repeat=1, blocks=ALL_BLOCKS):
    import concourse.bass as bass
    import concourse.bacc as bacc
    import concourse.mybir as mybir
    import concourse.tile as tile
    import concourse.bass_isa as bass_isa

    blocks = set(blocks)
    f32 = mybir.dt.float32
    bf16 = mybir.dt.bfloat16
    i32 = mybir.dt.int32
    u8 = mybir.dt.uint8
    Alu = mybir.AluOpType
    Act = mybir.ActivationFunctionType
    X = mybir.AxisListType.X

    nc = bacc.Bacc("TRN2", target_bir_lowering=False, debug=False)

    lg = nc.dram_tensor("logits", [BPC, N, NCH], f32, kind="ExternalInput")
    w1 = nc.dram_tensor("w1", [P, AT * 2], f32, kind="ExternalInput")
    c1 = nc.dram_tensor("c1", [P, AT * 2], f32, kind="ExternalInput")
    l1 = nc.dram_tensor("l1", [P, AT * 2], f32, kind="ExternalInput")
    wq = nc.dram_tensor("wq", [P, AT * CP], bf16, kind="ExternalInput")

    boxes_o = nc.dram_tensor("boxes", [BPC, N, 4], f32, kind="ExternalOutput")
    scores_o = nc.dram_tensor("scores", [BPC, N], f32, kind="ExternalOutput")
    classes_o = nc.dram_tensor("classes", [BPC, N], i32, kind="ExternalOutput")
    valid_o = nc.dram_tensor("valid", [BPC, N], u8, kind="ExternalOutput")
    num_o = nc.dram_tensor("num", [BPC], i32, kind="ExternalOutput")

    # DRAM views with anchor index n = p*APB + a
    lgv = lg.ap().rearrange("b (p a) c -> p b (a c)", p=P)          # [P,16,1850]
    bv = boxes_o.ap().rearrange("b (p a) c -> p b (a c)", p=P)      # [P,16,296]
    sv = scores_o.ap().rearrange("b (p a) -> p b a", p=P)           # [P,16,74]
    cv = classes_o.ap().rearrange("b (p a) -> p b a", p=P)
    vv = valid_o.ap().rearrange("b (p a) -> p b a", p=P)
    nv = num_o.ap().rearrange("(o b) -> o b", o=1)                  # [1,16]

    with tile.TileContext(nc) as tc:
        with (
            tc.tile_pool(name="consts", bufs=1) as consts,
            tc.tile_pool(name="lp", bufs=2) as lp,
            tc.tile_pool(name="eqp", bufs=2) as eqp,
            tc.tile_pool(name="wp", bufs=2) as wp,
        ):
            w1t = consts.tile([P, AT, 2], f32, tag="w1t")
            c1t = consts.tile([P, AT, 2], f32, tag="c1t")
            l1t = consts.tile([P, AT, 2], f32, tag="l1t")
            wqt = consts.tile([P, AT * CP], bf16, tag="wqt")
            nc.sync.dma_start(out=w1t[:].rearrange("p a c -> p (a c)"), in_=w1.ap())
            nc.sync.dma_start(out=c1t[:].rearrange("p a c -> p (a c)"), in_=c1.ap())
            nc.sync.dma_start(out=l1t[:].rearrange("p a c -> p (a c)"), in_=l1.ap())
            nc.sync.dma_start(out=wqt[:], in_=wq.ap())

            numpart = consts.tile([P, BPC], f32, tag="numpart")
            nc.vector.memset(numpart[:], 0.0)
            b21 = consts.tile([P, 1], f32, tag="b21")
            nc.vector.memset(b21[:], float(NCLS))

            # dummy store sources for disabled blocks (perf experiments only)
            if not {"scores", "argmax", "decode"} <= blocks:
                df32 = consts.tile([P, AT * 4], f32, tag="df32")
                di32 = consts.tile([P, AT], i32, tag="di32")
                du8 = consts.tile([P, AT], u8, tag="du8")
                nc.vector.memset(df32[:], 0.0)
                nc.vector.memset(di32[:], 0)
                nc.vector.memset(du8[:], 0)

            def st(view):
                return view.rearrange("p (b a) -> p b a", b=BPT)

            for _rep in range(repeat):
                for t in range(NTILES):
                    b0 = t * BPT
                    lt = lp.tile([P, BPT * APB * NCH], f32, tag="lt")
                    nc.sync.dma_start(
                        out=lt[:].rearrange("p (b x) -> p b x", b=BPT),
                        in_=lgv[:, b0 : b0 + BPT, :],
                    )
                    l3 = lt[:].rearrange("p (x c) -> p x c", c=NCH)  # [P,AT,25]
                    cls3 = l3[:, :, 4:NCH]                            # [P,AT,21]
                    loc01 = l3[:, :, 0:2]
                    loc23 = l3[:, :, 2:4]

                    # ---- scores = max over classes (f32, exact) ----
                    if "scores" in blocks:
                        m = wp.tile([P, AT], f32, tag="m")
                        nc.vector.reduce_max(out=m[:], in_=cls3, axis=X)
                        nc.sync.dma_start(out=sv[:, b0 : b0 + BPT, :], in_=st(m[:]))

                    # ---- argmax via eq * weight, first occurrence wins ----
                    if "argmax" in blocks:
                        eq = eqp.tile([P, AT, CP], bf16, tag="eq")
                        nc.gpsimd.memset(eq[:, :, NCLS:CP], 0.0)
                        m_ap = m[:]
                        mb = bass.AP(
                            tensor=m_ap.tensor,
                            offset=m_ap.offset,
                            ap=list(m_ap.ap) + [[0, NCLS]],
                        )  # [P, AT, 21] broadcast over class axis
                        nc.vector.tensor_tensor(
                            out=eq[:, :, 0:NCLS], in0=cls3, in1=mb, op=Alu.is_ge,
                        )
                        eqf = eq[:].rearrange("p a c -> p (a c)")
                        nc.vector.tensor_mul(eqf, eqf, wqt[:])
                        r = wp.tile([P, AT], bf16, tag="r")
                        nc.vector.reduce_max(out=r[:], in_=eq[:], axis=X)

                        ci = wp.tile([P, AT], i32, tag="ci")
                        nc.scalar.activation(
                            out=ci[:], in_=r[:], func=Act.Identity, bias=b21[:],
                            scale=-1.0,
                        )
                        nc.sync.dma_start(out=cv[:, b0 : b0 + BPT, :], in_=st(ci[:]))
                        vld = wp.tile([P, AT], u8, tag="vld")
                        for bb in range(BPT):
                            nc.scalar.activation(
                                out=vld[:, bb * APB : (bb + 1) * APB],
                                in_=r[:, bb * APB : (bb + 1) * APB],
                                func=Act.Sign, bias=b21[:], scale=-1.0,
                                accum_out=numpart[:, b0 + bb : b0 + bb + 1],
                            )
                        nc.sync.dma_start(out=vv[:, b0 : b0 + BPT, :], in_=st(vld[:]))
                    else:  # keep identical DMA traffic
                        nc.sync.dma_start(out=cv[:, b0 : b0 + BPT, :], in_=st(di32[:]))
                        nc.sync.dma_start(out=vv[:, b0 : b0 + BPT, :], in_=st(du8[:]))
                    if "scores" not in blocks:
                        nc.sync.dma_start(
                            out=sv[:, b0 : b0 + BPT, :], in_=st(df32[:, 0:AT])
                        )

                    # ---- box decode ----
                    if "decode" in blocks:
                        xy = wp.tile([P, AT, 2], f32, tag="xy")
                        nc.gpsimd.tensor_mul(xy[:], loc01, w1t[:])
                        nc.gpsimd.tensor_add(xy[:], xy[:], c1t[:])
                        bw = wp.tile([P, AT, 2], f32, tag="bw")
                        nc.scalar.activation(
                            out=bw[:].rearrange("p a c -> p (a c)"),
                            in_=loc23, func=Act.Exp,
                        )
                        nc.gpsimd.tensor_mul(bw[:], bw[:], l1t[:])
                        bx = wp.tile([P, AT, 4], f32, tag="bx")
                        nc.gpsimd.tensor_sub(bx[:, :, 0:2], xy[:], bw[:])
                        nc.gpsimd.tensor_add(bx[:, :, 2:4], xy[:], bw[:])
                        bxf = bx[:].rearrange("p a c -> p (a c)")
                        nc.vector.tensor_scalar(
                            out=bxf, in0=bxf, scalar1=0.0, scalar2=1.0,
                            op0=Alu.max, op1=Alu.min,
                        )
                        nc.sync.dma_start(
                            out=bv[:, b0 : b0 + BPT, :],
                            in_=bx[:].rearrange("p (b x) c -> p b (x c)", b=BPT),
                        )
                    else:
                        nc.sync.dma_start(
                            out=bv[:, b0 : b0 + BPT, :],
                            in_=df32[:].rearrange("p (b x) -> p b x", b=BPT),
                        )

            # ---- num = per-batch count over all partitions ----
            if "num" in blocks and "argmax" in blocks:
                allr = consts.tile([P, BPC], f32, tag="allr")
                nc.gpsimd.partition_all_reduce(
                    out_ap=allr[:], in_ap=numpart[:], channels=P,
                    reduce_op=bass_isa.ReduceOp.add,
                )
                numi = consts.tile([1, BPC], i32, tag="numi")
                nc.vector.tensor_copy(out=numi[:], in_=allr[0:1, :])
                nc.sync.dma_start(out=nv, in_=numi[:])
            else:
                numi = consts.tile([1, BPC], i32, tag="numi")
                nc.vector.memset(numi[:], 0)
                nc.sync.dma_start(out=nv, in_=numi[:])

    nc.compile()
    return nc


def host_tables(anchors):
    a = np.asarray(anchors, np.float32)
    cxcy = (a[:, 2:4] + a[:, 0:2]) * 0.5
    wh = a[:, 2:4] - a[:, 0:2]
    wh2 = wh * 0.5  # goes in the l1 slot: bwh/2 = exp(loc23) * wh/2

    def lay(t):  # [N,2] -> [P, AT*2], anchor n = p*APB + a, tiled over BPT
        t = t.reshape(P, APB, 2)
        return np.ascontiguousarray(
            np.tile(t[:, None], (1, BPT, 1, 1)).reshape(P, AT * 2)
        )

    wvals = np.array([NCLS - j for j in range(NCLS)] + [0], np.float32)
    wq = np.tile(wvals[None, None], (P, AT, 1)).reshape(P, AT * CP)
    return {
        "w1": lay(wh),
        "c1": lay(cxcy),
        "l1": lay(wh2),
        "wq": wq.astype(ml_dtypes.bfloat16),
    }


LAST_RESULT = None


def kernel(logits, anchors):
    global LAST_RESULT
    from concourse.bass_utils import run_bass_kernel_spmd

    if "nc" not in _CACHE:
        _CACHE["nc"] = build_program()
    nc = _CACHE["nc"]

    logits = np.ascontiguousarray(np.asarray(logits, np.float32))
    tables = host_tables(anchors)
    shards = logits.reshape(NCORES, BPC, N, NCH)
    in_maps = [
        {"logits": np.ascontiguousarray(shards[i]), **tables}
        for i in range(NCORES)
    ]
    res = run_bass_kernel_spmd(nc, in_maps, core_ids=list(range(NCORES)))
    LAST_RESULT = res

    boxes = np.concatenate([r["boxes"] for r in res.results], axis=0)
    scores = np.concatenate([r["scores"] for r in res.results], axis=0)
    classes = np.concatenate([r["classes"] for r in res.results], axis=0)
    valid = np.concatenate([r["valid"] for r in res.results], axis=0)
    num = np.concatenate([r["num"] for r in res.results], axis=0)
    return boxes, scores, classes.astype(np.int32), valid.astype(bool), num.astype(np.int32)
